# revision 24
# baseline (speedup 1.0000x reference)
"""Trainium2 Bass kernel for nn_Encoder (embedding -> LSTM scan with EOS
state-freezing, returns final (c, h) carry).

Key structural fact: the reference's EOS flag for a sequence is set from
``x[:, EOS_ID].astype(bool)`` where ``x`` is the *float* embedding row of the
current token.  A sequence's state therefore freezes permanently after the
first step whose token embedding has a nonzero feature at column EOS_ID.  The
host computes the exact number of scan steps ``T`` after which every
sequence is frozen (for randn-filled embeddings T == 1 with probability 1)
and the device only has to run those T steps.  For T == 1 the step
simplifies exactly (no approximation): h0 == c0 == 0, so the Wh matmul and
the forget gate contribute exactly nothing:

    gates = x0 @ Wx + b
    c = sigmoid(gates_i) * tanh(gates_g)
    h = sigmoid(gates_o) * tanh(c)

Sharding: the hidden dimension (and with it the i/g/o gate columns of Wx) is
split across the 8 cores, 64 hidden units each.  Each core computes its
[64 batch x 64 hidden] chunk of c and h on device (PE matmuls + Act LUT
sigmoid/tanh + DVE multiplies); the host concatenates the chunks into the
full [64, 512] outputs.

The host prepares the device inputs (weight layout, first-token embedding
rows in contraction-major order, bias row) exactly once per call; the device
program is a straight-line DMA-in -> matmul -> activation -> DMA-out with
every DMA shaped for large contiguous descriptors:

  blob [128, 1024] bf16, 2KB rows, transferred as three parallel streams
  (sync HWDGE, scalar HWDGE, gpsimd SWDGE) ordered so the i|g matmuls
  unblock chunk by chunk:
      cols [0:256)     xt:    xt[p, c*64+i]   = bf16(emb[tok_i, c*128+p])
      cols [256:768)   wx_ig: per chunk c the 128 i|g gate columns
      cols [768:1024)  wx_o:  per chunk c the 64 o gate columns
  aux  [1, 256] bf16 (bias_i|g|o row, then 64 ones), only when b != 0; a
      K=1 ones^T @ bias matmul broadcasts the bias into PSUM
  y    [64, 128] f32 (c | h) -> single output DMA (512B descriptors;
      bf16 output was measured slower: 256B descriptors pay the sub-512B
      read-modify-write DMA penalty)

The gate matmuls run as two PSUM accumulation groups (i|g then o) so the
sigmoid(i)/tanh(g) activations overlap the o-gate matmuls; sigmoid(o) is
written over the dead g-gate PSUM region, whose WAR hazard keeps it from
being scheduled ahead of tanh(g) on the scalar engine.
"""

import numpy as np

B, S, V, E, H = 64, 512, 32000, 512, 512
EOS_ID = 1
N_CORES = 8
HSH = H // N_CORES  # hidden slice per core: 64
G3 = 3 * HSH        # i/g/o gate columns per core: 192
KCH = E // 128      # contraction chunks: 4
XTW = KCH * B       # xt region cols in blob: 256
BLOBW = XTW + KCH * G3  # 1024

_cache = {}


def _sigmoid(x):
    return 1.0 / (1.0 + np.exp(-x))


def _lstm_numpy(inputs, embedding, Wx, Wh, b):
    """Faithful float32 fallback for the (probability ~0) case where not all
    sequences hit EOS on the first step."""
    Bn = inputs.shape[0]
    c = np.zeros((Bn, H), np.float32)
    h = np.zeros((Bn, H), np.float32)
    eos = np.zeros((Bn,), bool)
    for t in range(inputs.shape[1]):
        x = embedding[inputs[:, t]]
        g = x @ Wx + h @ Wh + b
        gi, gf, gg, go = np.split(g, 4, axis=1)
        new_c = _sigmoid(gf) * c + _sigmoid(gi) * np.tanh(gg)
        new_h = _sigmoid(go) * np.tanh(new_c)
        keep = eos[:, None]
        c = np.where(keep, c, new_c)
        h = np.where(keep, h, new_h)
        eos |= embedding[inputs[:, t], EOS_ID] != 0
        if eos.all():
            break
    return c, h


def _build_t1_program(has_bias):
    """One-step LSTM cell, gate-column sharded, batch-major gates, bf16."""
    import concourse.bacc as bacc
    import concourse.mybir as mybir
    import concourse.tile as tile

    f32 = mybir.dt.float32
    bf16 = mybir.dt.bfloat16
    nc = bacc.Bacc("TRN2", target_bir_lowering=False, debug=False,
                   num_devices=N_CORES, enable_partition_id=False)

    blob = nc.declare_dram_parameter("blob", [128, BLOBW], bf16,
                                     isOutput=False)
    if has_bias:
        aux = nc.declare_dram_parameter("aux", [1, G3 + B], bf16,
                                        isOutput=False)
    y = nc.declare_dram_parameter("y", [B, 2 * HSH], f32, isOutput=True)

    with tile.TileContext(nc) as tc:
        with (
            tc.tile_pool(name="sbuf", bufs=1) as sb,
            tc.tile_pool(name="psum", bufs=1, space="PSUM") as ps,
        ):
            # Input DMAs first.  Blob column order is [xt | wx_ig c0..c3 |
            # wx_o c0..c3] and the transfer is split three ways (two HWDGE
            # queues + an SWDGE stream from the otherwise-idle gpsimd) so
            # the i|g-gate matmuls unblock in chunk order.
            bl_sb = sb.tile([128, BLOBW], bf16, tag="blob")
            nc.sync.dma_start(bl_sb[:, 0:384], blob[:, 0:384])
            nc.scalar.dma_start(bl_sb[:, 384:640], blob[:, 384:640])
            nc.gpsimd.dma_start(bl_sb[:, 640:BLOBW], blob[:, 640:BLOBW])
            if has_bias:
                aux_sb = sb.tile([1, G3 + B], bf16, tag="aux")
                nc.sync.dma_start(aux_sb[:], aux[:])

            # gates = bias + sum_c xt_c^T @ wx_c, in two PSUM groups: the
            # i|g half first so sigmoid(i)/tanh(g) overlap the o-half
            # matmuls.  With a nonzero bias a K=1 matmul (ones^T @ bias row)
            # opens each group; with b == 0 (the always case here) the first
            # gate matmul opens it and nothing waits on the aux DMA.
            IGW = 2 * HSH
            gig = ps.tile([B, IGW], f32, tag="gig")
            go = ps.tile([B, HSH], f32, tag="go")
            if has_bias:
                nc.tensor.matmul(gig[:], lhsT=aux_sb[0:1, G3:G3 + B],
                                 rhs=aux_sb[0:1, 0:IGW], start=True,
                                 stop=False)
            for c in range(KCH):
                nc.tensor.matmul(
                    gig[:], lhsT=bl_sb[:, c * B:(c + 1) * B],
                    rhs=bl_sb[:, XTW + c * IGW:XTW + (c + 1) * IGW],
                    start=(c == 0 and not has_bias), stop=(c == KCH - 1))
            OBASE = XTW + KCH * IGW  # 768
            if has_bias:
                nc.tensor.matmul(go[:], lhsT=aux_sb[0:1, G3:G3 + B],
                                 rhs=aux_sb[0:1, IGW:G3], start=True,
                                 stop=False)
            for c in range(KCH):
                nc.tensor.matmul(
                    go[:], lhsT=bl_sb[:, c * B:(c + 1) * B],
                    rhs=bl_sb[:, OBASE + c * HSH:OBASE + (c + 1) * HSH],
                    start=(c == 0 and not has_bias), stop=(c == KCH - 1))

            Act = mybir.ActivationFunctionType
            y_sb = sb.tile([B, 2 * HSH], f32, tag="y")
            sig_i = sb.tile([B, HSH], f32, tag="sig_i")
            nc.scalar.activation(sig_i[:], gig[:, 0:HSH], Act.Sigmoid)
            tanh_g = sb.tile([B, HSH], f32, tag="tanh_g")
            nc.scalar.activation(tanh_g[:], gig[:, HSH:2 * HSH], Act.Tanh)
            # sigmoid(o) overwrites the dead g-gate PSUM region: the WAR
            # hazard pins it after tanh_g in the scalar engine's order, so
            # the tanh_g -> mul_c critical chain is never delayed behind it.
            nc.scalar.activation(gig[:, HSH:2 * HSH], go[:], Act.Sigmoid)
            nc.vector.tensor_mul(y_sb[:, 0:HSH], sig_i[:], tanh_g[:])
            tanh_c = sb.tile([B, HSH], f32, tag="tanh_c")
            nc.scalar.activation(tanh_c[:], y_sb[:, 0:HSH], Act.Tanh)
            nc.vector.tensor_mul(y_sb[:, HSH:2 * HSH], gig[:, HSH:2 * HSH],
                                 tanh_c[:])
            nc.sync.dma_start(y[:], y_sb[:])

    nc.compile()
    return nc


def _make_in_maps(inputs, embedding, Wx, b, has_bias):
    import concourse.mybir as mybir

    np_bf16 = mybir.dt.np(mybir.dt.bfloat16)

    # Per-core static blocks (wx layout + aux row), cached across calls for
    # the same Wx/b arrays (the cache holds references, so identity is safe).
    static = _cache.get("static")
    if (static is None or _cache.get("static_wx") is not Wx
            or _cache.get("static_b") is not b):
        wx_list, aux_list = [], []
        for k in range(N_CORES):
            sl = slice(k * HSH, (k + 1) * HSH)
            # gate columns of Wx for this core: i, g, o (f unused: c0 == 0)
            wx_k = np.concatenate(
                [Wx[:, 0 * H:1 * H][:, sl], Wx[:, 2 * H:3 * H][:, sl],
                 Wx[:, 3 * H:4 * H][:, sl]], axis=1)  # [E, G3]
            # [E, G3] -> [KCH, 128, G3] -> [128, KCH*G3]
            wx_k = np.ascontiguousarray(
                wx_k.reshape(KCH, 128, G3).transpose(1, 0, 2).reshape(
                    128, KCH * G3).astype(np_bf16))
            brow = np.concatenate(
                [b[0 * H:1 * H][sl], b[2 * H:3 * H][sl], b[3 * H:4 * H][sl],
                 np.ones((B,), np.float32)])
            aux_list.append(np.ascontiguousarray(
                brow.astype(np_bf16).reshape(1, G3 + B)))
            wx_list.append(wx_k)
        static = (wx_list, aux_list)
        _cache["static"] = static
        _cache["static_wx"] = Wx
        _cache["static_b"] = b
    wx_list, aux_list = static

    # First-token embedding rows, bf16, contraction-major:
    # xt[p, c*64 + i] = emb[tok_i, c*128 + p]
    x = embedding[inputs[:, 0]].astype(np_bf16)          # [B, E]
    xt = np.ascontiguousarray(
        x.T.reshape(KCH, 128, B).transpose(1, 0, 2).reshape(128, XTW))

    in_maps = []
    for k in range(N_CORES):
        wx3 = wx_list[k].reshape(128, KCH, G3)
        blob = np.concatenate(
            [xt, wx3[:, :, 0:2 * HSH].reshape(128, KCH * 2 * HSH),
             wx3[:, :, 2 * HSH:G3].reshape(128, KCH * HSH)],
            axis=1)  # [128, BLOBW]
        m = {"blob": np.ascontiguousarray(blob)}
        if has_bias:
            m["aux"] = aux_list[k]
        in_maps.append(m)
    return in_maps


def _unpack_results(results):
    c = np.empty((B, H), np.float32)
    h = np.empty((B, H), np.float32)
    for k in range(N_CORES):
        sl = slice(k * HSH, (k + 1) * HSH)
        yk = results[k]["y"].astype(np.float32)
        c[:, sl] = yk[:, 0:HSH]
        h[:, sl] = yk[:, HSH:2 * HSH]
    return c, h


def _run_t1(inputs, embedding, Wx, b):
    from concourse.bass_utils import run_bass_kernel_spmd

    has_bias = bool(np.any(b))
    key = ("t1", has_bias)
    if key not in _cache:
        _cache[key] = _build_t1_program(has_bias)
    nc = _cache[key]
    in_maps = _make_in_maps(inputs, embedding, Wx, b, has_bias)
    res = run_bass_kernel_spmd(nc, in_maps, core_ids=list(range(N_CORES)))
    return _unpack_results(res.results)


def kernel(inputs, embedding, Wx, Wh, b):
    inputs = np.asarray(inputs)
    embedding = np.asarray(embedding, dtype=np.float32)
    Wx = np.asarray(Wx, dtype=np.float32)
    Wh = np.asarray(Wh, dtype=np.float32)
    b = np.asarray(b, dtype=np.float32)

    # Exact host-side computation of how many scan steps can change state:
    # sequence b freezes forever after its first step with
    # embedding[token, EOS_ID] != 0.
    eos = np.zeros((inputs.shape[0],), bool)
    T = 0
    for t in range(inputs.shape[1]):
        eos |= embedding[inputs[:, t], EOS_ID] != 0
        T = t + 1
        if eos.all():
            break

    if T == 1:
        return _run_t1(inputs, embedding, Wx, b)
    # Probability-zero fallback (an embedding value exactly 0.0 at EOS_ID).
    return _lstm_numpy(inputs, embedding, Wx, Wh, b)


# revision 26
# speedup vs baseline: 1.0170x; 1.0170x over previous
"""Trainium2 Bass kernel for nn_Encoder (embedding -> LSTM scan with EOS
state-freezing, returns final (c, h) carry).

Key structural fact: the reference's EOS flag for a sequence is set from
``x[:, EOS_ID].astype(bool)`` where ``x`` is the *float* embedding row of the
current token.  A sequence's state therefore freezes permanently after the
first step whose token embedding has a nonzero feature at column EOS_ID.  The
host computes the exact number of scan steps ``T`` after which every
sequence is frozen (for randn-filled embeddings T == 1 with probability 1)
and the device only has to run those T steps.  For T == 1 the step
simplifies exactly (no approximation): h0 == c0 == 0, so the Wh matmul and
the forget gate contribute exactly nothing:

    gates = x0 @ Wx + b
    c = sigmoid(gates_i) * tanh(gates_g)
    h = sigmoid(gates_o) * tanh(c)

Sharding: the hidden dimension (and with it the i/g/o gate columns of Wx) is
split across the 8 cores, 64 hidden units each.  Each core computes its
[64 batch x 64 hidden] chunk of c and h on device (PE matmuls + Act LUT
sigmoid/tanh + DVE multiplies); the host concatenates the chunks into the
full [64, 512] outputs.

The host prepares the device inputs (weight layout, first-token embedding
rows in contraction-major order, bias row) exactly once per call; the device
program is a straight-line DMA-in -> matmul -> activation -> DMA-out with
every DMA shaped for large contiguous descriptors:

  blob [128, 1024] bf16, 2KB rows, transferred as three parallel streams
  (sync HWDGE, scalar HWDGE, gpsimd SWDGE) ordered so the i|g matmuls
  unblock chunk by chunk:
      per chunk c (blocks of 192 cols): [xt_c (64) | wx_ig_c (128)] with
      xt[p, i] = bf16(emb[tok_i, c*128+p]); then cols [768:1024) the four
      64-col o-gate blocks.  Chunks 0-1 ride the sync queue, 2-3 the
      scalar queue, o-gates the SWDGE stream, so all four i|g matmuls
      become ready together at the two HWDGE semaphores
  aux  [1, 256] bf16 (bias_i|g|o row, then 64 ones), only when b != 0; a
      K=1 ones^T @ bias matmul broadcasts the bias into PSUM
  y    [64, 128] f32 (c | h) -> single output DMA (512B descriptors;
      bf16 output was measured slower: 256B descriptors pay the sub-512B
      read-modify-write DMA penalty)

The gate matmuls run as two PSUM accumulation groups (i|g then o) so the
sigmoid(i)/tanh(g) activations overlap the o-gate matmuls; sigmoid(o) is
written over the dead g-gate PSUM region, whose WAR hazard keeps it from
being scheduled ahead of tanh(g) on the scalar engine.
"""

import numpy as np

B, S, V, E, H = 64, 512, 32000, 512, 512
EOS_ID = 1
N_CORES = 8
HSH = H // N_CORES  # hidden slice per core: 64
G3 = 3 * HSH        # i/g/o gate columns per core: 192
KCH = E // 128      # contraction chunks: 4
XTW = KCH * B       # xt region cols in blob: 256
BLOBW = XTW + KCH * G3  # 1024

_cache = {}


def _sigmoid(x):
    return 1.0 / (1.0 + np.exp(-x))


def _lstm_numpy(inputs, embedding, Wx, Wh, b):
    """Faithful float32 fallback for the (probability ~0) case where not all
    sequences hit EOS on the first step."""
    Bn = inputs.shape[0]
    c = np.zeros((Bn, H), np.float32)
    h = np.zeros((Bn, H), np.float32)
    eos = np.zeros((Bn,), bool)
    for t in range(inputs.shape[1]):
        x = embedding[inputs[:, t]]
        g = x @ Wx + h @ Wh + b
        gi, gf, gg, go = np.split(g, 4, axis=1)
        new_c = _sigmoid(gf) * c + _sigmoid(gi) * np.tanh(gg)
        new_h = _sigmoid(go) * np.tanh(new_c)
        keep = eos[:, None]
        c = np.where(keep, c, new_c)
        h = np.where(keep, h, new_h)
        eos |= embedding[inputs[:, t], EOS_ID] != 0
        if eos.all():
            break
    return c, h


def _build_t1_program(has_bias):
    """One-step LSTM cell, gate-column sharded, batch-major gates, bf16."""
    import concourse.bacc as bacc
    import concourse.mybir as mybir
    import concourse.tile as tile

    f32 = mybir.dt.float32
    bf16 = mybir.dt.bfloat16
    nc = bacc.Bacc("TRN2", target_bir_lowering=False, debug=False,
                   num_devices=N_CORES, enable_partition_id=False)

    blob = nc.declare_dram_parameter("blob", [128, BLOBW], bf16,
                                     isOutput=False)
    if has_bias:
        aux = nc.declare_dram_parameter("aux", [1, G3 + B], bf16,
                                        isOutput=False)
    y = nc.declare_dram_parameter("y", [B, 2 * HSH], f32, isOutput=True)

    with tile.TileContext(nc) as tc:
        with (
            tc.tile_pool(name="sbuf", bufs=1) as sb,
            tc.tile_pool(name="psum", bufs=1, space="PSUM") as ps,
        ):
            # Input DMAs first.  Blob column order is [xt | wx_ig c0..c3 |
            # wx_o c0..c3] and the transfer is split three ways (two HWDGE
            # queues + an SWDGE stream from the otherwise-idle gpsimd) so
            # the i|g-gate matmuls unblock in chunk order.
            bl_sb = sb.tile([128, BLOBW], bf16, tag="blob")
            nc.sync.dma_start(bl_sb[:, 0:384], blob[:, 0:384])
            nc.scalar.dma_start(bl_sb[:, 384:768], blob[:, 384:768])
            nc.gpsimd.dma_start(bl_sb[:, 768:BLOBW], blob[:, 768:BLOBW])
            if has_bias:
                aux_sb = sb.tile([1, G3 + B], bf16, tag="aux")
                nc.sync.dma_start(aux_sb[:], aux[:])

            # gates = bias + sum_c xt_c^T @ wx_c, in two PSUM groups: the
            # i|g half first so sigmoid(i)/tanh(g) overlap the o-half
            # matmuls.  With a nonzero bias a K=1 matmul (ones^T @ bias row)
            # opens each group; with b == 0 (the always case here) the first
            # gate matmul opens it and nothing waits on the aux DMA.
            IGW = 2 * HSH
            gig = ps.tile([B, IGW], f32, tag="gig")
            go = ps.tile([B, HSH], f32, tag="go")
            if has_bias:
                nc.tensor.matmul(gig[:], lhsT=aux_sb[0:1, G3:G3 + B],
                                 rhs=aux_sb[0:1, 0:IGW], start=True,
                                 stop=False)
            CW = B + IGW  # 192: per-chunk [xt_c | ig_c] block width
            for c in range(KCH):
                nc.tensor.matmul(
                    gig[:], lhsT=bl_sb[:, c * CW:c * CW + B],
                    rhs=bl_sb[:, c * CW + B:(c + 1) * CW],
                    start=(c == 0 and not has_bias), stop=(c == KCH - 1))
            OBASE = KCH * CW  # 768
            if has_bias:
                nc.tensor.matmul(go[:], lhsT=aux_sb[0:1, G3:G3 + B],
                                 rhs=aux_sb[0:1, IGW:G3], start=True,
                                 stop=False)
            for c in range(KCH):
                nc.tensor.matmul(
                    go[:], lhsT=bl_sb[:, c * CW:c * CW + B],
                    rhs=bl_sb[:, OBASE + c * HSH:OBASE + (c + 1) * HSH],
                    start=(c == 0 and not has_bias), stop=(c == KCH - 1))

            Act = mybir.ActivationFunctionType
            y_sb = sb.tile([B, 2 * HSH], f32, tag="y")
            sig_i = sb.tile([B, HSH], f32, tag="sig_i")
            nc.scalar.activation(sig_i[:], gig[:, 0:HSH], Act.Sigmoid)
            tanh_g = sb.tile([B, HSH], f32, tag="tanh_g")
            nc.scalar.activation(tanh_g[:], gig[:, HSH:2 * HSH], Act.Tanh)
            # sigmoid(o) overwrites the dead g-gate PSUM region: the WAR
            # hazard pins it after tanh_g in the scalar engine's order, so
            # the tanh_g -> mul_c critical chain is never delayed behind it.
            nc.scalar.activation(gig[:, HSH:2 * HSH], go[:], Act.Sigmoid)
            nc.vector.tensor_mul(y_sb[:, 0:HSH], sig_i[:], tanh_g[:])
            tanh_c = sb.tile([B, HSH], f32, tag="tanh_c")
            nc.scalar.activation(tanh_c[:], y_sb[:, 0:HSH], Act.Tanh)
            nc.vector.tensor_mul(y_sb[:, HSH:2 * HSH], gig[:, HSH:2 * HSH],
                                 tanh_c[:])
            nc.sync.dma_start(y[:], y_sb[:])

    nc.compile()
    return nc


def _make_in_maps(inputs, embedding, Wx, b, has_bias):
    import concourse.mybir as mybir

    np_bf16 = mybir.dt.np(mybir.dt.bfloat16)

    # Per-core static blocks (wx layout + aux row), cached across calls for
    # the same Wx/b arrays (the cache holds references, so identity is safe).
    static = _cache.get("static")
    if (static is None or _cache.get("static_wx") is not Wx
            or _cache.get("static_b") is not b):
        wx_list, aux_list = [], []
        for k in range(N_CORES):
            sl = slice(k * HSH, (k + 1) * HSH)
            # gate columns of Wx for this core: i, g, o (f unused: c0 == 0)
            wx_k = np.concatenate(
                [Wx[:, 0 * H:1 * H][:, sl], Wx[:, 2 * H:3 * H][:, sl],
                 Wx[:, 3 * H:4 * H][:, sl]], axis=1)  # [E, G3]
            # [E, G3] -> [KCH, 128, G3] -> [128, KCH*G3]
            wx_k = np.ascontiguousarray(
                wx_k.reshape(KCH, 128, G3).transpose(1, 0, 2).reshape(
                    128, KCH * G3).astype(np_bf16))
            brow = np.concatenate(
                [b[0 * H:1 * H][sl], b[2 * H:3 * H][sl], b[3 * H:4 * H][sl],
                 np.ones((B,), np.float32)])
            aux_list.append(np.ascontiguousarray(
                brow.astype(np_bf16).reshape(1, G3 + B)))
            wx_list.append(wx_k)
        static = (wx_list, aux_list)
        _cache["static"] = static
        _cache["static_wx"] = Wx
        _cache["static_b"] = b
    wx_list, aux_list = static

    # First-token embedding rows, bf16, contraction-major:
    # xt[p, c*64 + i] = emb[tok_i, c*128 + p]
    x = embedding[inputs[:, 0]].astype(np_bf16)          # [B, E]
    xt = np.ascontiguousarray(
        x.T.reshape(KCH, 128, B).transpose(1, 0, 2).reshape(128, XTW))

    in_maps = []
    for k in range(N_CORES):
        wx3 = wx_list[k].reshape(128, KCH, G3)
        parts = []
        for c in range(KCH):
            parts.append(xt[:, c * B:(c + 1) * B])       # xt_c
            parts.append(wx3[:, c, 0:2 * HSH])           # ig_c
        parts.append(wx3[:, :, 2 * HSH:G3].reshape(128, KCH * HSH))  # o
        blob = np.concatenate(parts, axis=1)  # [128, BLOBW]
        m = {"blob": np.ascontiguousarray(blob)}
        if has_bias:
            m["aux"] = aux_list[k]
        in_maps.append(m)
    return in_maps


def _unpack_results(results):
    c = np.empty((B, H), np.float32)
    h = np.empty((B, H), np.float32)
    for k in range(N_CORES):
        sl = slice(k * HSH, (k + 1) * HSH)
        yk = results[k]["y"].astype(np.float32)
        c[:, sl] = yk[:, 0:HSH]
        h[:, sl] = yk[:, HSH:2 * HSH]
    return c, h


def _run_t1(inputs, embedding, Wx, b):
    from concourse.bass_utils import run_bass_kernel_spmd

    has_bias = bool(np.any(b))
    key = ("t1", has_bias)
    if key not in _cache:
        _cache[key] = _build_t1_program(has_bias)
    nc = _cache[key]
    in_maps = _make_in_maps(inputs, embedding, Wx, b, has_bias)
    res = run_bass_kernel_spmd(nc, in_maps, core_ids=list(range(N_CORES)))
    return _unpack_results(res.results)


def kernel(inputs, embedding, Wx, Wh, b):
    inputs = np.asarray(inputs)
    embedding = np.asarray(embedding, dtype=np.float32)
    Wx = np.asarray(Wx, dtype=np.float32)
    Wh = np.asarray(Wh, dtype=np.float32)
    b = np.asarray(b, dtype=np.float32)

    # Exact host-side computation of how many scan steps can change state:
    # sequence b freezes forever after its first step with
    # embedding[token, EOS_ID] != 0.
    eos = np.zeros((inputs.shape[0],), bool)
    T = 0
    for t in range(inputs.shape[1]):
        eos |= embedding[inputs[:, t], EOS_ID] != 0
        T = t + 1
        if eos.all():
            break

    if T == 1:
        return _run_t1(inputs, embedding, Wx, b)
    # Probability-zero fallback (an embedding value exactly 0.0 at EOS_ID).
    return _lstm_numpy(inputs, embedding, Wx, Wh, b)
